# revision 8
# baseline (speedup 1.0000x reference)
"""DeepSets segment-reduce kernel for 8x TRN2 NeuronCores (Bass/Tile).

Computes: out = rho_mlp(segment_mean(phi_mlp(ins), batch))  for
sorted segment ids `batch` in [0, 50000), ins [1M, 128] f32.

Strategy:
  - Segments are grouped in windows of 128, windows assigned contiguously
    to the 8 cores (no segment straddles a core => zero cross-core
    collectives). One SPMD NEFF serves all cores; per-window row counts
    are padded to a uniform block count (pad rows contribute exactly 0).
  - Host preprocessing (numpy): transpose ins per window into fp16
    [128, slots] tiles with each row pre-scaled by 1/count(segment) (this
    folds the segment-mean into the phi layer-1 output, and zeroes pad
    rows), plus fp8 one-hot selection matrices S per 128-row block.
  - Device, per window:
      h1 = relu(Xs @ W1 + rinv*b1)   -- X^T block stationary, W1 moving;
                                        bias via K=4 block-diag rank-1
      T' = sum_b h1r_b^T-contracted  -- matmul lhsT=h1r_b, rhs=S_b (fp8)
           => T'[hid, seg] accumulated in PSUM = segment-MEANS of h1r
      seg_mean^T = W2^T @ T' + b2 x nz  (phi layer 2 reassociated onto
           50k segments instead of 1M rows, ~20x less work)
      rho MLP on seg_mean^T per window ([128,128] matmuls), biases via
      per-partition ACT bias in the transposed orientation.
  - All intermediates fp16 (full PE rate, values O(1)); end-to-end
    scale-relative absmax vs fp32 reference ~5e-4.

kernel(**inputs) takes the full unsharded inputs and returns the full
[50000, 128] fp32 output.
"""

import numpy as np
import ml_dtypes

import concourse.mybir as mybir
import concourse.tile as tile
from concourse import bacc
from concourse.bass_utils import run_bass_kernel_spmd

P = 128
N_CORES = 8
F16 = mybir.dt.float16
F32 = mybir.dt.float32
F8 = mybir.dt.float8e4
FP8NP = ml_dtypes.float8_e4m3


def _f16(a):
    return np.asarray(a, dtype=np.float32).astype(np.float16)


def _host_prep(ins, batch, wts, n_segs):
    """Shard rows by 128-segment windows; build per-core device arrays."""
    batch = np.asarray(batch).astype(np.int64)
    nwin_tot = -(-n_segs // P)                      # 391 for 50000
    nw = -(-nwin_tot // N_CORES)                    # windows per core (49)
    nwin_pad = nw * N_CORES                         # 392
    segs_pad = nwin_pad * P                         # 50176

    counts = np.bincount(batch, minlength=segs_pad).astype(np.float64)
    invc = np.where(counts > 0, 1.0 / np.maximum(counts, 1.0), 0.0)
    nz = (counts > 0).astype(np.float16)

    bounds = np.searchsorted(batch, np.arange(nwin_pad + 1) * P, side="left")
    win_cnt = np.diff(bounds)
    nb = max(1, int(-(-win_cnt.max() // P)))        # 128-row blocks per window
    slots = nb * P
    nch = -(-nb // 4)                               # psum chunks per window

    row_invc = invc[batch].astype(np.float32)
    ins = np.asarray(ins, dtype=np.float32)
    seg_ar = np.arange(P, dtype=np.int64)

    per_core = []
    for c in range(N_CORES):
        xt = np.zeros((P, nw * slots), dtype=np.float16)
        relpad = np.full((nw, slots), -1, dtype=np.int64)
        rinvpad = np.zeros((nw, slots), dtype=np.float32)
        for w in range(nw):
            g = c * nw + w
            s, e = bounds[g], bounds[g + 1]
            cnt = e - s
            if cnt == 0:
                continue
            # fp16 X^T, pre-scaled by 1/count so the segment reduction
            # directly produces means (pad rows scale to exactly 0)
            xt[:, w * slots : w * slots + cnt] = \
                (ins[s:e] * row_invc[s:e, None]).T
            relpad[w, :cnt] = batch[s:e] - g * P
            rinvpad[w, :cnt] = row_invc[s:e]
        # fp8 one-hot S per block: S[r, s] = (rel[r] == s)
        oneh = (relpad.reshape(nw * nb, P)[:, :, None] == seg_ar)  # [blk,r,s]
        sfp8 = np.ascontiguousarray(
            oneh.transpose(1, 0, 2).reshape(P, nw * nb * P)
        ).astype(FP8NP)
        # K=4 rank-1 bias weights: rinv rows per chunk-block
        rinv4 = np.zeros((4, nw * nch * P), dtype=np.float16)
        r3 = rinvpad.reshape(nw, nb, P)
        for ci in range(nch):
            csz = min(4, nb - ci * 4)
            for k in range(csz):
                col = (np.arange(nw) * nch + ci)[:, None] * P + np.arange(P)
                rinv4[k, col.ravel()] = r3[:, ci * 4 + k, :].astype(
                    np.float16).ravel()
        nz2 = np.ascontiguousarray(np.broadcast_to(
            nz[c * nw * P : (c + 1) * nw * P], (2, nw * P)))
        per_core.append({"xt": xt, "sfp8": sfp8, "rinv4": rinv4, "nz2": nz2})
    return per_core, nw, nb


def _host_consts(wts):
    b1 = _f16(wts["phi_b1"])
    b1diag = np.zeros((4, 512), dtype=np.float16)
    for k in range(4):
        b1diag[k, k * P : (k + 1) * P] = b1
    b2hi = _f16(wts["phi_b2"])
    b2lo = _f16(np.asarray(wts["phi_b2"], np.float32) - b2hi.astype(np.float32))
    return {
        "w1": _f16(wts["phi_W1"]),
        "w2": _f16(wts["phi_W2"]),
        "rw1": _f16(wts["rho_W1"]),
        "rw2": _f16(wts["rho_W2"]),
        "b1diag": b1diag,
        "b2c2": np.stack([b2hi, b2lo]),
        "rb1": np.asarray(wts["rho_b1"], np.float32).reshape(P, 1),
        "rb2": np.asarray(wts["rho_b2"], np.float32).reshape(P, 1),
    }


def _build(nw, nb, consts_np):
    """Emit the SPMD single-core program (same NEFF for all 8 cores)."""
    slots = nb * P
    nch = -(-nb // 4)
    nc = bacc.Bacc("TRN2", target_bir_lowering=False, debug=False,
                   num_devices=N_CORES)

    d_xt = nc.dram_tensor("xt", [P, nw * slots], F16, kind="ExternalInput").ap()
    d_s = nc.dram_tensor("sfp8", [P, nw * nb * P], F8,
                         kind="ExternalInput").ap()
    d_r4 = nc.dram_tensor("rinv4", [4, nw * nch * P], F16,
                          kind="ExternalInput").ap()
    d_nz2 = nc.dram_tensor("nz2", [2, nw * P], F16, kind="ExternalInput").ap()
    d_consts = {
        k: nc.dram_tensor(
            k, list(v.shape), mybir.dt.from_np(v.dtype), kind="ExternalInput"
        ).ap()
        for k, v in consts_np.items()
    }
    d_out = nc.dram_tensor("outT", [P, nw * P], F32, kind="ExternalOutput").ap()

    chunks = []
    off = 0
    while off < nb:
        cs = min(4, nb - off)
        chunks.append((off, cs))
        off += cs

    with tile.TileContext(nc) as tc:
        with (
            tc.tile_pool(name="const", bufs=1) as constp,
            tc.tile_pool(name="outsb", bufs=1) as outp,
            tc.tile_pool(name="xt", bufs=3) as xtp,
            tc.tile_pool(name="sfp", bufs=3) as sfpp,
            tc.tile_pool(name="h1r", bufs=2) as h1rp,
            tc.tile_pool(name="tail16", bufs=6) as tailp,
            tc.tile_pool(name="h1ps", bufs=3, space="PSUM") as h1psp,
            tc.tile_pool(name="tps", bufs=2, space="PSUM") as tpsp,
            tc.tile_pool(name="tailps", bufs=2, space="PSUM") as tailpsp,
        ):
            cs_ = {}
            for k, v in consts_np.items():
                cs_[k] = constp.tile(
                    list(v.shape), mybir.dt.from_np(v.dtype), name=f"c_{k}"
                )
                nc.sync.dma_start(cs_[k], d_consts[k])
            r4sb = constp.tile([4, nw * nch * P], F16)
            nc.sync.dma_start(r4sb, d_r4)
            nz2sb = constp.tile([2, nw * P], F16)
            nc.sync.dma_start(nz2sb, d_nz2)
            outsb = outp.tile([P, nw * P], F32)

            # groups of up to 4 windows share one phi-L2 + rho tail (N=512
            # matmuls amortize the serial per-window tail latency)
            GW = 4
            for w0 in range(0, nw, GW):
                g = min(GW, nw - w0)
                t_sb = tailp.tile([P, g * P], F16, tag="t_sb",
                                  padded_shape=[P, GW * P])
                for w in range(w0, w0 + g):
                    xt = xtp.tile([P, slots], F16)
                    nc.sync.dma_start(xt, d_xt[:, w * slots : (w + 1) * slots])
                    st = sfpp.tile([P, nb * P], F8)
                    nc.sync.dma_start(
                        st, d_s[:, w * nb * P : (w + 1) * nb * P])

                    # ---- phi layer 1: h1r = relu(Xs @ W1 + rinv*b1)
                    h1r = h1rp.tile([P, slots], F16)
                    for ci, (coff, csz) in enumerate(chunks):
                        h1ps = h1psp.tile([P, 512], F32, space="PSUM",
                                          tag="h1ps")
                        reg = h1ps[:, : csz * P]
                        c4 = (w * nch + ci) * P
                        nc.tensor.matmul(
                            reg, lhsT=r4sb[:, c4 : c4 + P],
                            rhs=cs_["b1diag"][:, : csz * P],
                            start=True, stop=False,
                        )
                        for j in range(csz):
                            b = coff + j
                            nc.tensor.matmul(
                                h1ps[:, j * P : (j + 1) * P],
                                lhsT=xt[:, b * P : (b + 1) * P],
                                rhs=cs_["w1"],
                                start=False, stop=(j == csz - 1),
                            )
                        dst = h1r[:, coff * P : (coff + csz) * P]
                        if ci % 2 == 0:
                            nc.scalar.activation(
                                dst, reg, mybir.ActivationFunctionType.Relu)
                        else:
                            nc.vector.tensor_scalar(
                                dst, reg, 0.0, None, op0=mybir.AluOpType.max)

                    # ---- segment-mean reduce: T'[hid,seg] += h1r_b^T @ S_b
                    tps = tpsp.tile([P, P], F32, space="PSUM", tag="tps")
                    for b in range(nb):
                        nc.tensor.matmul(
                            tps, lhsT=h1r[:, b * P : (b + 1) * P],
                            rhs=st[:, b * P : (b + 1) * P],
                            start=(b == 0), stop=(b == nb - 1),
                        )
                    nc.scalar.copy(t_sb[:, (w - w0) * P : (w - w0 + 1) * P],
                                   tps)

                # ---- phi layer 2 on segment means (whole group):
                #      sm^T = W2^T @ T' + b2 x nz
                smps = tailpsp.tile([P, g * P], F32, space="PSUM",
                                    tag="tailps", padded_shape=[P, GW * P])
                nc.tensor.matmul(smps, lhsT=cs_["w2"], rhs=t_sb,
                                 start=True, stop=False)
                nc.tensor.matmul(
                    smps, lhsT=cs_["b2c2"],
                    rhs=nz2sb[:, w0 * P : (w0 + g) * P],
                    start=False, stop=True,
                )
                sm_sb = tailp.tile([P, g * P], F16, tag="sm_sb",
                                   padded_shape=[P, GW * P])
                nc.vector.tensor_copy(sm_sb, smps)

                # ---- rho MLP (feature-major: per-partition ACT biases)
                r1ps = tailpsp.tile([P, g * P], F32, space="PSUM",
                                    tag="tailps", padded_shape=[P, GW * P])
                nc.tensor.matmul(r1ps, lhsT=cs_["rw1"], rhs=sm_sb,
                                 start=True, stop=True)
                r1_sb = tailp.tile([P, g * P], F16, tag="r1_sb",
                                   padded_shape=[P, GW * P])
                nc.scalar.activation(
                    r1_sb, r1ps, mybir.ActivationFunctionType.Relu,
                    bias=cs_["rb1"][:, :1],
                )
                ops_ = tailpsp.tile([P, g * P], F32, space="PSUM",
                                    tag="tailps", padded_shape=[P, GW * P])
                nc.tensor.matmul(ops_, lhsT=cs_["rw2"], rhs=r1_sb,
                                 start=True, stop=True)
                nc.scalar.activation(
                    outsb[:, w0 * P : (w0 + g) * P], ops_,
                    mybir.ActivationFunctionType.Identity,
                    bias=cs_["rb2"][:, :1],
                )

            nc.sync.dma_start(d_out, outsb)

    nc.compile()
    return nc


def _run(inputs, n_segs=50000, trace=False, **hw_kwargs):
    ins = np.asarray(inputs["ins"])
    batch = np.asarray(inputs["batch"])
    per_core, nw, nb = _host_prep(ins, batch, inputs, n_segs)
    consts_np = _host_consts(inputs)
    nc = _build(nw, nb, consts_np)

    in_maps = []
    for c in range(N_CORES):
        m = dict(consts_np)
        m.update(per_core[c])
        in_maps.append(m)
    res = run_bass_kernel_spmd(
        nc, in_maps, core_ids=list(range(N_CORES)), trace=trace, **hw_kwargs
    )
    outs = [r["outT"] for r in res.results]             # [128, nw*128] f32
    full = np.concatenate([o.T for o in outs], axis=0)  # [8*nw*128, 128]
    return np.ascontiguousarray(full[:n_segs]), res


def kernel(**inputs):
    out, _ = _run(inputs)
    return out


# revision 9
# speedup vs baseline: 1.4142x; 1.4142x over previous
"""DeepSets segment-reduce kernel for 8x TRN2 NeuronCores (Bass/Tile).

Computes: out = rho_mlp(segment_mean(phi_mlp(ins), batch))  for
sorted segment ids `batch` in [0, 50000), ins [1M, 128] f32.

Strategy:
  - Segments are grouped in windows of 128, windows assigned contiguously
    to the 8 cores (no segment straddles a core => zero cross-core
    collectives). One SPMD NEFF serves all cores; per-window row counts
    are padded to a uniform block count (pad rows contribute exactly 0).
  - Host preprocessing (numpy): transpose ins per window into fp16
    [128, slots] tiles with each row pre-scaled by 1/count(segment) (this
    folds the segment-mean into the phi layer-1 output, and zeroes pad
    rows), plus fp8 one-hot selection matrices S per 128-row block.
  - Device, per window:
      h1 = relu(Xs @ W1 + rinv*b1)   -- X^T block stationary, W1 moving;
                                        bias via K=4 block-diag rank-1
      T' = sum_b h1r_b^T-contracted  -- matmul lhsT=h1r_b, rhs=S_b (fp8)
           => T'[hid, seg] accumulated in PSUM = segment-MEANS of h1r
      seg_mean^T = W2^T @ T' + b2 x nz  (phi layer 2 reassociated onto
           50k segments instead of 1M rows, ~20x less work)
      rho MLP on seg_mean^T per window ([128,128] matmuls), biases via
      per-partition ACT bias in the transposed orientation.
  - All intermediates fp16 (full PE rate, values O(1)); end-to-end
    scale-relative absmax vs fp32 reference ~5e-4.

kernel(**inputs) takes the full unsharded inputs and returns the full
[50000, 128] fp32 output.
"""

import numpy as np
import ml_dtypes

import concourse.mybir as mybir
import concourse.tile as tile
from concourse import bacc
from concourse.bass_utils import run_bass_kernel_spmd

P = 128
N_CORES = 8
F16 = mybir.dt.float16
F32 = mybir.dt.float32
F8 = mybir.dt.float8e4
FP8NP = ml_dtypes.float8_e4m3


def _f16(a):
    return np.asarray(a, dtype=np.float32).astype(np.float16)


def _host_prep(ins, batch, wts, n_segs):
    """Shard rows by 128-segment windows; build per-core device arrays."""
    batch = np.asarray(batch).astype(np.int64)
    nwin_tot = -(-n_segs // P)                      # 391 for 50000
    nw = -(-nwin_tot // N_CORES)                    # windows per core (49)
    nwin_pad = nw * N_CORES                         # 392
    segs_pad = nwin_pad * P                         # 50176

    counts = np.bincount(batch, minlength=segs_pad).astype(np.float64)
    invc = np.where(counts > 0, 1.0 / np.maximum(counts, 1.0), 0.0)
    nz = (counts > 0).astype(np.float16)

    bounds = np.searchsorted(batch, np.arange(nwin_pad + 1) * P, side="left")
    win_cnt = np.diff(bounds)
    nb = max(1, int(-(-win_cnt.max() // P)))        # 128-row blocks per window
    slots = nb * P
    nch = -(-nb // 4)                               # psum chunks per window

    row_invc = invc[batch].astype(np.float32)
    ins = np.asarray(ins, dtype=np.float32)
    seg_ar = np.arange(P, dtype=np.int64)

    per_core = []
    for c in range(N_CORES):
        xt = np.zeros((P, nw * slots), dtype=np.float16)
        relpad = np.full((nw, slots), -1, dtype=np.int64)
        rinvpad = np.zeros((nw, slots), dtype=np.float32)
        for w in range(nw):
            g = c * nw + w
            s, e = bounds[g], bounds[g + 1]
            cnt = e - s
            if cnt == 0:
                continue
            # fp16 X^T, pre-scaled by 1/count so the segment reduction
            # directly produces means (pad rows scale to exactly 0)
            xt[:, w * slots : w * slots + cnt] = \
                (ins[s:e] * row_invc[s:e, None]).T
            relpad[w, :cnt] = batch[s:e] - g * P
            rinvpad[w, :cnt] = row_invc[s:e]
        # fp8 one-hot S per block: S[r, s] = (rel[r] == s)
        oneh = (relpad.reshape(nw * nb, P)[:, :, None] == seg_ar)  # [blk,r,s]
        sfp8 = np.ascontiguousarray(
            oneh.transpose(1, 0, 2).reshape(P, nw * nb * P)
        ).astype(FP8NP)
        # K=4 rank-1 bias weights: rinv rows per chunk-block
        rinv4 = np.zeros((4, nw * nch * P), dtype=np.float16)
        r3 = rinvpad.reshape(nw, nb, P)
        for ci in range(nch):
            csz = min(4, nb - ci * 4)
            for k in range(csz):
                col = (np.arange(nw) * nch + ci)[:, None] * P + np.arange(P)
                rinv4[k, col.ravel()] = r3[:, ci * 4 + k, :].astype(
                    np.float16).ravel()
        nz2 = np.ascontiguousarray(np.broadcast_to(
            nz[c * nw * P : (c + 1) * nw * P], (2, nw * P)))
        per_core.append({"xt": xt, "sfp8": sfp8, "rinv4": rinv4, "nz2": nz2})
    return per_core, nw, nb


def _host_consts(wts):
    b1 = _f16(wts["phi_b1"])
    b1diag = np.zeros((4, 512), dtype=np.float16)
    for k in range(4):
        b1diag[k, k * P : (k + 1) * P] = b1
    b2hi = _f16(wts["phi_b2"])
    b2lo = _f16(np.asarray(wts["phi_b2"], np.float32) - b2hi.astype(np.float32))
    return {
        "w1": _f16(wts["phi_W1"]),
        "w2": _f16(wts["phi_W2"]),
        "rw1": _f16(wts["rho_W1"]),
        "rw2": _f16(wts["rho_W2"]),
        "b1diag": b1diag,
        "b2c2": np.stack([b2hi, b2lo]),
        "rb1": np.asarray(wts["rho_b1"], np.float32).reshape(P, 1),
        "rb2": np.asarray(wts["rho_b2"], np.float32).reshape(P, 1),
    }


def _build(nw, nb, consts_np):
    """Emit the SPMD single-core program (same NEFF for all 8 cores)."""
    slots = nb * P
    nch = -(-nb // 4)
    nc = bacc.Bacc("TRN2", target_bir_lowering=False, debug=False,
                   num_devices=N_CORES)

    d_xt = nc.dram_tensor("xt", [P, nw * slots], F16, kind="ExternalInput").ap()
    d_s = nc.dram_tensor("sfp8", [P, nw * nb * P], F8,
                         kind="ExternalInput").ap()
    d_r4 = nc.dram_tensor("rinv4", [4, nw * nch * P], F16,
                          kind="ExternalInput").ap()
    d_nz2 = nc.dram_tensor("nz2", [2, nw * P], F16, kind="ExternalInput").ap()
    d_consts = {
        k: nc.dram_tensor(
            k, list(v.shape), mybir.dt.from_np(v.dtype), kind="ExternalInput"
        ).ap()
        for k, v in consts_np.items()
    }
    d_out = nc.dram_tensor("outT", [P, nw * P], F32, kind="ExternalOutput").ap()

    chunks = []
    off = 0
    while off < nb:
        cs = min(4, nb - off)
        chunks.append((off, cs))
        off += cs

    with tile.TileContext(nc) as tc:
        with (
            tc.tile_pool(name="const", bufs=1) as constp,
            tc.tile_pool(name="outsb", bufs=1) as outp,
            tc.tile_pool(name="xt", bufs=3) as xtp,
            tc.tile_pool(name="sfp", bufs=3) as sfpp,
            tc.tile_pool(name="h1r", bufs=2) as h1rp,
            tc.tile_pool(name="tail16", bufs=6) as tailp,
            tc.tile_pool(name="h1ps", bufs=3, space="PSUM") as h1psp,
            tc.tile_pool(name="tps", bufs=2, space="PSUM") as tpsp,
            tc.tile_pool(name="tailps", bufs=2, space="PSUM") as tailpsp,
        ):
            cs_ = {}
            for k, v in consts_np.items():
                cs_[k] = constp.tile(
                    list(v.shape), mybir.dt.from_np(v.dtype), name=f"c_{k}"
                )
                nc.sync.dma_start(cs_[k], d_consts[k])
            r4sb = constp.tile([4, nw * nch * P], F16)
            nc.sync.dma_start(r4sb, d_r4)
            nz2sb = constp.tile([2, nw * P], F16)
            nc.sync.dma_start(nz2sb, d_nz2)
            outsb = outp.tile([P, nw * P], F32)

            # The K=4 bias rank-1 for each psum chunk is emitted one chunk
            # AHEAD of the block matmuls that accumulate onto it: a matmul
            # that accumulates onto the region a previous matmul just wrote
            # pays a full drain-before-fill bubble (~320ns); with >=4
            # unrelated matmuls in between the drain is hidden.
            def emit_bias(w, ci):
                coff, csz = chunks[ci]
                t = h1psp.tile([P, 512], F32, space="PSUM", tag="h1ps",
                               name=f"h1ps_{w}_{ci}")
                c4 = (w * nch + ci) * P
                nc.tensor.matmul(
                    t[:, : csz * P], lhsT=r4sb[:, c4 : c4 + P],
                    rhs=cs_["b1diag"][:, : csz * P],
                    start=True, stop=False,
                )
                return t

            all_chunks = [(w, ci) for w in range(nw) for ci in range(nch)]
            gpos = 0
            nxt_tile = emit_bias(*all_chunks[0])

            # groups of up to 4 windows share one phi-L2 + rho tail (N=512
            # matmuls amortize the serial per-window tail latency)
            GW = 4
            for w0 in range(0, nw, GW):
                g = min(GW, nw - w0)
                t_sb = tailp.tile([P, g * P], F16, tag="t_sb",
                                  padded_shape=[P, GW * P])
                for w in range(w0, w0 + g):
                    xt = xtp.tile([P, slots], F16)
                    nc.sync.dma_start(xt, d_xt[:, w * slots : (w + 1) * slots])
                    st = sfpp.tile([P, nb * P], F8)
                    nc.sync.dma_start(
                        st, d_s[:, w * nb * P : (w + 1) * nb * P])

                    # ---- phi layer 1: h1r = relu(Xs @ W1 + rinv*b1)
                    h1r = h1rp.tile([P, slots], F16)
                    for ci, (coff, csz) in enumerate(chunks):
                        h1ps = nxt_tile
                        if gpos + 1 < len(all_chunks):
                            nxt_tile = emit_bias(*all_chunks[gpos + 1])
                        gpos += 1
                        reg = h1ps[:, : csz * P]
                        for j in range(csz):
                            b = coff + j
                            nc.tensor.matmul(
                                h1ps[:, j * P : (j + 1) * P],
                                lhsT=xt[:, b * P : (b + 1) * P],
                                rhs=cs_["w1"],
                                start=False, stop=(j == csz - 1),
                            )
                        dst = h1r[:, coff * P : (coff + csz) * P]
                        if ci % 2 == 0:
                            nc.scalar.activation(
                                dst, reg, mybir.ActivationFunctionType.Relu)
                        else:
                            nc.vector.tensor_scalar(
                                dst, reg, 0.0, None, op0=mybir.AluOpType.max)

                    # ---- segment-mean reduce: T'[hid,seg] += h1r_b^T @ S_b
                    tps = tpsp.tile([P, P], F32, space="PSUM", tag="tps")
                    for b in range(nb):
                        nc.tensor.matmul(
                            tps, lhsT=h1r[:, b * P : (b + 1) * P],
                            rhs=st[:, b * P : (b + 1) * P],
                            start=(b == 0), stop=(b == nb - 1),
                        )
                    nc.scalar.copy(t_sb[:, (w - w0) * P : (w - w0 + 1) * P],
                                   tps)

                # ---- phi layer 2 on segment means (whole group):
                #      sm^T = W2^T @ T' + b2 x nz
                smps = tailpsp.tile([P, g * P], F32, space="PSUM",
                                    tag="tailps", padded_shape=[P, GW * P])
                nc.tensor.matmul(smps, lhsT=cs_["w2"], rhs=t_sb,
                                 start=True, stop=False)
                nc.tensor.matmul(
                    smps, lhsT=cs_["b2c2"],
                    rhs=nz2sb[:, w0 * P : (w0 + g) * P],
                    start=False, stop=True,
                )
                sm_sb = tailp.tile([P, g * P], F16, tag="sm_sb",
                                   padded_shape=[P, GW * P])
                nc.vector.tensor_copy(sm_sb, smps)

                # ---- rho MLP (feature-major: per-partition ACT biases)
                r1ps = tailpsp.tile([P, g * P], F32, space="PSUM",
                                    tag="tailps", padded_shape=[P, GW * P])
                nc.tensor.matmul(r1ps, lhsT=cs_["rw1"], rhs=sm_sb,
                                 start=True, stop=True)
                r1_sb = tailp.tile([P, g * P], F16, tag="r1_sb",
                                   padded_shape=[P, GW * P])
                nc.scalar.activation(
                    r1_sb, r1ps, mybir.ActivationFunctionType.Relu,
                    bias=cs_["rb1"][:, :1],
                )
                ops_ = tailpsp.tile([P, g * P], F32, space="PSUM",
                                    tag="tailps", padded_shape=[P, GW * P])
                nc.tensor.matmul(ops_, lhsT=cs_["rw2"], rhs=r1_sb,
                                 start=True, stop=True)
                nc.scalar.activation(
                    outsb[:, w0 * P : (w0 + g) * P], ops_,
                    mybir.ActivationFunctionType.Identity,
                    bias=cs_["rb2"][:, :1],
                )

            nc.sync.dma_start(d_out, outsb)

    nc.compile()
    return nc


def _run(inputs, n_segs=50000, trace=False, **hw_kwargs):
    ins = np.asarray(inputs["ins"])
    batch = np.asarray(inputs["batch"])
    per_core, nw, nb = _host_prep(ins, batch, inputs, n_segs)
    consts_np = _host_consts(inputs)
    nc = _build(nw, nb, consts_np)

    in_maps = []
    for c in range(N_CORES):
        m = dict(consts_np)
        m.update(per_core[c])
        in_maps.append(m)
    res = run_bass_kernel_spmd(
        nc, in_maps, core_ids=list(range(N_CORES)), trace=trace, **hw_kwargs
    )
    outs = [r["outT"] for r in res.results]             # [128, nw*128] f32
    full = np.concatenate([o.T for o in outs], axis=0)  # [8*nw*128, 128]
    return np.ascontiguousarray(full[:n_segs]), res


def kernel(**inputs):
    out, _ = _run(inputs)
    return out


# revision 13
# speedup vs baseline: 1.7441x; 1.2333x over previous
"""DeepSets segment-reduce kernel for 8x TRN2 NeuronCores (Bass/Tile).

Computes: out = rho_mlp(segment_mean(phi_mlp(ins), batch))  for
sorted segment ids `batch` in [0, 50000), ins [1M, 128] f32.

Strategy:
  - Segments are grouped in windows of 128, windows assigned contiguously
    to the 8 cores (no segment straddles a core => zero cross-core
    collectives). One SPMD NEFF serves all cores; per-window row counts
    are padded to a uniform block count (pad rows contribute exactly 0).
  - Host preprocessing (numpy): transpose ins per window into fp16
    [128, slots] tiles with each row pre-scaled by 1/count(segment) (this
    folds the segment-mean into the phi layer-1 output, and zeroes pad
    rows), plus fp8 one-hot selection matrices S per 128-row block.
  - Device, per window:
      h1 = relu(Xs @ W1 + rinv*b1)   -- X^T block stationary, W1 moving;
                                        bias via K=4 block-diag rank-1
      T' = sum_b h1r_b^T-contracted  -- matmul lhsT=h1r_b, rhs=S_b (fp8)
           => T'[hid, seg] accumulated in PSUM = segment-MEANS of h1r
      seg_mean^T = W2^T @ T' + b2 x nz  (phi layer 2 reassociated onto
           50k segments instead of 1M rows, ~20x less work)
      rho MLP on seg_mean^T per window ([128,128] matmuls), biases via
      per-partition ACT bias in the transposed orientation.
  - All intermediates fp16 (full PE rate, values O(1)); end-to-end
    scale-relative absmax vs fp32 reference ~5e-4.

kernel(**inputs) takes the full unsharded inputs and returns the full
[50000, 128] fp32 output.
"""

import numpy as np
import ml_dtypes

import concourse.mybir as mybir
import concourse.tile as tile
from concourse import bacc
from concourse.bass_utils import run_bass_kernel_spmd

P = 128
N_CORES = 8
F16 = mybir.dt.float16
F32 = mybir.dt.float32
F8 = mybir.dt.float8e4
FP8NP = ml_dtypes.float8_e4m3


def _f16(a):
    return np.asarray(a, dtype=np.float32).astype(np.float16)


def _host_prep(ins, batch, wts, n_segs):
    """Shard rows by 128-segment windows; build per-core device arrays."""
    batch = np.asarray(batch).astype(np.int64)
    nwin_tot = -(-n_segs // P)                      # 391 for 50000
    nw = -(-nwin_tot // N_CORES)                    # windows per core (49)
    nwin_pad = nw * N_CORES                         # 392
    segs_pad = nwin_pad * P                         # 50176

    counts = np.bincount(batch, minlength=segs_pad).astype(np.float64)
    invc = np.where(counts > 0, 1.0 / np.maximum(counts, 1.0), 0.0)
    nz = (counts > 0).astype(np.float16)

    bounds = np.searchsorted(batch, np.arange(nwin_pad + 1) * P, side="left")
    win_cnt = np.diff(bounds)
    nb = max(1, int(-(-win_cnt.max() // P)))        # 128-row blocks per window
    slots = nb * P
    nch = -(-nb // 4)                               # psum chunks per window

    row_invc = invc[batch].astype(np.float32)
    ins = np.asarray(ins, dtype=np.float32)
    seg_ar = np.arange(P, dtype=np.int64)

    per_core = []
    for c in range(N_CORES):
        xt = np.zeros((P, nw * slots), dtype=np.float16)
        relpad = np.full((nw, slots), -1, dtype=np.int64)
        rinvpad = np.zeros((nw, slots), dtype=np.float32)
        for w in range(nw):
            g = c * nw + w
            s, e = bounds[g], bounds[g + 1]
            cnt = e - s
            if cnt == 0:
                continue
            # fp16 X^T, pre-scaled by 1/count so the segment reduction
            # directly produces means (pad rows scale to exactly 0)
            xt[:, w * slots : w * slots + cnt] = \
                (ins[s:e] * row_invc[s:e, None]).T
            relpad[w, :cnt] = batch[s:e] - g * P
            rinvpad[w, :cnt] = row_invc[s:e]
        # fp8 one-hot S per block: S[r, s] = (rel[r] == s)
        oneh = (relpad.reshape(nw * nb, P)[:, :, None] == seg_ar)  # [blk,r,s]
        sfp8 = np.ascontiguousarray(
            oneh.transpose(1, 0, 2).reshape(P, nw * nb * P)
        ).astype(FP8NP)
        # K=4 rank-1 bias weights: rinv rows per chunk-block
        rinv4 = np.zeros((4, nw * nch * P), dtype=np.float16)
        r3 = rinvpad.reshape(nw, nb, P)
        for ci in range(nch):
            csz = min(4, nb - ci * 4)
            for k in range(csz):
                col = (np.arange(nw) * nch + ci)[:, None] * P + np.arange(P)
                rinv4[k, col.ravel()] = r3[:, ci * 4 + k, :].astype(
                    np.float16).ravel()
        nz2 = np.ascontiguousarray(np.broadcast_to(
            nz[c * nw * P : (c + 1) * nw * P], (2, nw * P)))
        per_core.append({"xt": xt, "sfp8": sfp8, "rinv4": rinv4, "nz2": nz2})
    return per_core, nw, nb


def _host_consts(wts):
    b1 = _f16(wts["phi_b1"])
    # K=128 block-diagonal bias rhs: rows 0-3 carry b1 per chunk-block,
    # rows 4-127 are zero (so the lhsT rows 4-127 can hold garbage)
    b1diag = np.zeros((P, 512), dtype=np.float16)
    for k in range(4):
        b1diag[k, k * P : (k + 1) * P] = b1
    b2hi = _f16(wts["phi_b2"])
    b2lo = _f16(np.asarray(wts["phi_b2"], np.float32) - b2hi.astype(np.float32))
    cpack16 = np.concatenate(
        [_f16(wts["phi_W1"]), _f16(wts["phi_W2"]),
         _f16(wts["rho_W1"]), _f16(wts["rho_W2"]), b1diag], axis=1)
    cpack32 = np.concatenate(
        [np.asarray(wts["rho_b1"], np.float32).reshape(P, 1),
         np.asarray(wts["rho_b2"], np.float32).reshape(P, 1)], axis=1)
    return {
        "cpack16": cpack16,          # [128, 1024]: w1|w2|rw1|rw2|b1diag
        "cpack32": cpack32,          # [128, 2]:    rb1|rb2
        "b2c2": np.stack([b2hi, b2lo]),
    }


def _build(nw, nb, consts_np):
    """Emit the SPMD single-core program (same NEFF for all 8 cores)."""
    slots = nb * P
    nch = -(-nb // 4)
    nc = bacc.Bacc("TRN2", target_bir_lowering=False, debug=False,
                   num_devices=N_CORES)

    d_xt = nc.dram_tensor("xt", [P, nw * slots], F16, kind="ExternalInput").ap()
    d_s = nc.dram_tensor("sfp8", [P, nw * nb * P], F8,
                         kind="ExternalInput").ap()
    d_r4 = nc.dram_tensor("rinv4", [4, nw * nch * P], F16,
                          kind="ExternalInput").ap()
    d_nz2 = nc.dram_tensor("nz2", [2, nw * P], F16, kind="ExternalInput").ap()
    d_consts = {
        k: nc.dram_tensor(
            k, list(v.shape), mybir.dt.from_np(v.dtype), kind="ExternalInput"
        ).ap()
        for k, v in consts_np.items()
    }
    d_out = nc.dram_tensor("outT", [P, nw * P], F32, kind="ExternalOutput").ap()

    chunks = []
    off = 0
    while off < nb:
        cs = min(4, nb - off)
        chunks.append((off, cs))
        off += cs

    with tile.TileContext(nc) as tc:
        with (
            tc.tile_pool(name="const", bufs=1) as constp,
            tc.tile_pool(name="outsb", bufs=1) as outp,
            tc.tile_pool(name="xt", bufs=3) as xtp,
            tc.tile_pool(name="sfp", bufs=3) as sfpp,
            tc.tile_pool(name="h1r", bufs=2) as h1rp,
            tc.tile_pool(name="tail16", bufs=6) as tailp,
            tc.tile_pool(name="h1ps", bufs=3, space="PSUM") as h1psp,
            tc.tile_pool(name="tps", bufs=2, space="PSUM") as tpsp,
            tc.tile_pool(name="tailps", bufs=2, space="PSUM") as tailpsp,
        ):
            cs_ = {}
            for k, v in consts_np.items():
                cs_[k] = constp.tile(
                    list(v.shape), mybir.dt.from_np(v.dtype), name=f"c_{k}"
                )
                # scalar-engine HWDGE ring: parallel with the sync-ring
                # window DMAs so startup isn't serialized
                nc.scalar.dma_start(cs_[k], d_consts[k])
            w1_c = cs_["cpack16"][:, 0:128]
            w2_c = cs_["cpack16"][:, 128:256]
            rw1_c = cs_["cpack16"][:, 256:384]
            rw2_c = cs_["cpack16"][:, 384:512]
            b1diag_c = cs_["cpack16"][:, 512:1024]
            rb1_c = cs_["cpack32"][:, 0:1]
            rb2_c = cs_["cpack32"][:, 1:2]
            r4sb = constp.tile([4, nw * nch * P], F16)
            nc.scalar.dma_start(r4sb, d_r4)
            nz2sb = constp.tile([2, nw * P], F16)
            nc.scalar.dma_start(nz2sb, d_nz2)
            outsb = outp.tile([P, nw * P], F32)
            # ping-pong K=128 stationary for the bias rank-1s: rows 0-3 get
            # the per-window rinv values (gpsimd copy, one window ahead),
            # rows 4-127 are zeroed once and multiply b1diag's zero rows
            padt = constp.tile([P, 2 * nch * P], F16, name="padt")
            nc.gpsimd.memset(padt, 0.0)

            def emit_rinv_copy(w):
                half = (w % 2) * nch * P
                nc.gpsimd.tensor_copy(
                    padt[0:4, half : half + nch * P],
                    r4sb[:, w * nch * P : (w + 1) * nch * P])

            # The K=4 bias rank-1 for each psum chunk is emitted one chunk
            # AHEAD of the block matmuls that accumulate onto it: a matmul
            # that accumulates onto the region a previous matmul just wrote
            # pays a full drain-before-fill bubble (~320ns); with >=4
            # unrelated matmuls in between the drain is hidden.
            def emit_bias(w, ci):
                coff, csz = chunks[ci]
                t = h1psp.tile([P, 512], F32, space="PSUM", tag="h1ps",
                               name=f"h1ps_{w}_{ci}")
                half = (w % 2) * nch * P
                nc.tensor.matmul(
                    t[:, : csz * P],
                    lhsT=padt[:, half + ci * P : half + (ci + 1) * P],
                    rhs=b1diag_c[:, : csz * P],
                    start=True, stop=False,
                )
                return t

            all_chunks = [(w, ci) for w in range(nw) for ci in range(nch)]
            gpos = 0
            emit_rinv_copy(0)
            nxt_tile = emit_bias(*all_chunks[0])

            # groups of up to 4 windows share one phi-L2 + rho tail (N=512
            # matmuls amortize the serial per-window tail latency)
            GW = 4
            for w0 in range(0, nw, GW):
                g = min(GW, nw - w0)
                t_sb = tailp.tile([P, g * P], F16, tag="t_sb",
                                  padded_shape=[P, GW * P])
                for w in range(w0, w0 + g):
                    if w + 1 < nw:
                        emit_rinv_copy(w + 1)
                    xt = xtp.tile([P, slots], F16)
                    nc.sync.dma_start(xt, d_xt[:, w * slots : (w + 1) * slots])
                    st = sfpp.tile([P, nb * P], F8)
                    nc.sync.dma_start(
                        st, d_s[:, w * nb * P : (w + 1) * nb * P])

                    # ---- phi layer 1: h1r = relu(Xs @ W1 + rinv*b1)
                    h1r = h1rp.tile([P, slots], F16)
                    for ci, (coff, csz) in enumerate(chunks):
                        h1ps = nxt_tile
                        if gpos + 1 < len(all_chunks):
                            nxt_tile = emit_bias(*all_chunks[gpos + 1])
                        gpos += 1
                        reg = h1ps[:, : csz * P]
                        for j in range(csz):
                            b = coff + j
                            nc.tensor.matmul(
                                h1ps[:, j * P : (j + 1) * P],
                                lhsT=xt[:, b * P : (b + 1) * P],
                                rhs=w1_c,
                                start=False, stop=(j == csz - 1),
                            )
                        dst = h1r[:, coff * P : (coff + csz) * P]
                        if ci % 2 == 0:
                            nc.scalar.activation(
                                dst, reg, mybir.ActivationFunctionType.Relu)
                        else:
                            nc.vector.tensor_scalar(
                                dst, reg, 0.0, None, op0=mybir.AluOpType.max)

                    # ---- segment-mean reduce: T'[hid,seg] += h1r_b^T @ S_b
                    tps = tpsp.tile([P, P], F32, space="PSUM", tag="tps")
                    for b in range(nb):
                        nc.tensor.matmul(
                            tps, lhsT=h1r[:, b * P : (b + 1) * P],
                            rhs=st[:, b * P : (b + 1) * P],
                            start=(b == 0), stop=(b == nb - 1),
                        )
                    nc.scalar.copy(t_sb[:, (w - w0) * P : (w - w0 + 1) * P],
                                   tps)

                # ---- phi layer 2 on segment means (whole group):
                #      sm^T = W2^T @ T' + b2 x nz
                smps = tailpsp.tile([P, g * P], F32, space="PSUM",
                                    tag="tailps", padded_shape=[P, GW * P])
                nc.tensor.matmul(smps, lhsT=w2_c, rhs=t_sb,
                                 start=True, stop=False)
                nc.tensor.matmul(
                    smps, lhsT=cs_["b2c2"],
                    rhs=nz2sb[:, w0 * P : (w0 + g) * P],
                    start=False, stop=True,
                )
                sm_sb = tailp.tile([P, g * P], F16, tag="sm_sb",
                                   padded_shape=[P, GW * P])
                nc.vector.tensor_copy(sm_sb, smps)

                # ---- rho MLP (feature-major: per-partition ACT biases)
                r1ps = tailpsp.tile([P, g * P], F32, space="PSUM",
                                    tag="tailps", padded_shape=[P, GW * P])
                nc.tensor.matmul(r1ps, lhsT=rw1_c, rhs=sm_sb,
                                 start=True, stop=True)
                r1_sb = tailp.tile([P, g * P], F16, tag="r1_sb",
                                   padded_shape=[P, GW * P])
                nc.scalar.activation(
                    r1_sb, r1ps, mybir.ActivationFunctionType.Relu,
                    bias=rb1_c,
                )
                ops_ = tailpsp.tile([P, g * P], F32, space="PSUM",
                                    tag="tailps", padded_shape=[P, GW * P])
                nc.tensor.matmul(ops_, lhsT=rw2_c, rhs=r1_sb,
                                 start=True, stop=True)
                nc.scalar.activation(
                    outsb[:, w0 * P : (w0 + g) * P], ops_,
                    mybir.ActivationFunctionType.Identity,
                    bias=rb2_c,
                )
                nc.sync.dma_start(d_out[:, w0 * P : (w0 + g) * P],
                                  outsb[:, w0 * P : (w0 + g) * P])

    nc.compile()
    return nc


def _run(inputs, n_segs=50000, trace=False, **hw_kwargs):
    ins = np.asarray(inputs["ins"])
    batch = np.asarray(inputs["batch"])
    per_core, nw, nb = _host_prep(ins, batch, inputs, n_segs)
    consts_np = _host_consts(inputs)
    nc = _build(nw, nb, consts_np)

    in_maps = []
    for c in range(N_CORES):
        m = dict(consts_np)
        m.update(per_core[c])
        in_maps.append(m)
    res = run_bass_kernel_spmd(
        nc, in_maps, core_ids=list(range(N_CORES)), trace=trace, **hw_kwargs
    )
    outs = [r["outT"] for r in res.results]             # [128, nw*128] f32
    full = np.concatenate([o.T for o in outs], axis=0)  # [8*nw*128, 128]
    return np.ascontiguousarray(full[:n_segs]), res


def kernel(**inputs):
    out, _ = _run(inputs)
    return out


# revision 15
# speedup vs baseline: 1.8487x; 1.0600x over previous
"""DeepSets segment-reduce kernel for 8x TRN2 NeuronCores (Bass/Tile).

Computes: out = rho_mlp(segment_mean(phi_mlp(ins), batch))  for
sorted segment ids `batch` in [0, 50000), ins [1M, 128] f32.

Strategy:
  - Segments are grouped in windows of 128, windows assigned contiguously
    to the 8 cores (no segment straddles a core => zero cross-core
    collectives). One SPMD NEFF serves all cores; per-window row counts
    are padded to a uniform block count (pad rows contribute exactly 0).
  - Host preprocessing (numpy): transpose ins per window into fp16
    [128, slots] tiles with each row pre-scaled by 1/count(segment) (this
    folds the segment-mean into the phi layer-1 output, and zeroes pad
    rows), plus fp8 one-hot selection matrices S per 128-row block.
  - Device, per window:
      h1 = relu(Xs @ W1 + rinv*b1)   -- X^T block stationary, W1 moving;
                                        bias via K=4 block-diag rank-1
      T' = sum_b h1r_b^T-contracted  -- matmul lhsT=h1r_b, rhs=S_b (fp8)
           => T'[hid, seg] accumulated in PSUM = segment-MEANS of h1r
      seg_mean^T = W2^T @ T' + b2 x nz  (phi layer 2 reassociated onto
           50k segments instead of 1M rows, ~20x less work)
      rho MLP on seg_mean^T per window ([128,128] matmuls), biases via
      per-partition ACT bias in the transposed orientation.
  - All intermediates fp16 (full PE rate, values O(1)); end-to-end
    scale-relative absmax vs fp32 reference ~5e-4.

kernel(**inputs) takes the full unsharded inputs and returns the full
[50000, 128] fp32 output.
"""

import numpy as np
import ml_dtypes

import concourse.mybir as mybir
import concourse.tile as tile
from concourse import bacc
from concourse.bass_utils import run_bass_kernel_spmd

P = 128
N_CORES = 8
F16 = mybir.dt.float16
F32 = mybir.dt.float32
F8 = mybir.dt.float8e4
FP8NP = ml_dtypes.float8_e4m3


def _f16(a):
    return np.asarray(a, dtype=np.float32).astype(np.float16)


def _host_prep(ins, batch, wts, n_segs):
    """Shard rows by 128-segment windows; build per-core device arrays."""
    batch = np.asarray(batch).astype(np.int64)
    nwin_tot = -(-n_segs // P)                      # 391 for 50000
    nw = -(-nwin_tot // N_CORES)                    # windows per core (49)
    nwin_pad = nw * N_CORES                         # 392
    segs_pad = nwin_pad * P                         # 50176

    counts = np.bincount(batch, minlength=segs_pad).astype(np.float64)
    invc = np.where(counts > 0, 1.0 / np.maximum(counts, 1.0), 0.0)
    nz = (counts > 0).astype(np.float16)

    bounds = np.searchsorted(batch, np.arange(nwin_pad + 1) * P, side="left")
    win_cnt = np.diff(bounds)
    nb = max(1, int(-(-win_cnt.max() // P)))        # 128-row blocks per window
    slots = nb * P
    nch = -(-nb // 4)                               # psum chunks per window

    row_invc = invc[batch].astype(np.float32)
    ins = np.asarray(ins, dtype=np.float32)
    seg_ar = np.arange(P, dtype=np.int64)

    per_core = []
    for c in range(N_CORES):
        xt = np.zeros((P, nw * slots), dtype=np.float16)
        relpad = np.full((nw, slots), -1, dtype=np.int64)
        rinvpad = np.zeros((nw, slots), dtype=np.float32)
        for w in range(nw):
            g = c * nw + w
            s, e = bounds[g], bounds[g + 1]
            cnt = e - s
            if cnt == 0:
                continue
            # fp16 X^T, pre-scaled by 1/count so the segment reduction
            # directly produces means (pad rows scale to exactly 0)
            xt[:, w * slots : w * slots + cnt] = \
                (ins[s:e] * row_invc[s:e, None]).T
            relpad[w, :cnt] = batch[s:e] - g * P
            rinvpad[w, :cnt] = row_invc[s:e]
        # fp8 one-hot S per block: S[r, s] = (rel[r] == s)
        oneh = (relpad.reshape(nw * nb, P)[:, :, None] == seg_ar)  # [blk,r,s]
        sfp8 = np.ascontiguousarray(
            oneh.transpose(1, 0, 2).reshape(P, nw * nb * P)
        ).astype(FP8NP)
        # K=4 rank-1 bias weights: rinv rows per chunk-block
        rinv4 = np.zeros((4, nw * nch * P), dtype=np.float16)
        r3 = rinvpad.reshape(nw, nb, P)
        for ci in range(nch):
            csz = min(4, nb - ci * 4)
            for k in range(csz):
                col = (np.arange(nw) * nch + ci)[:, None] * P + np.arange(P)
                rinv4[k, col.ravel()] = r3[:, ci * 4 + k, :].astype(
                    np.float16).ravel()
        nz2 = np.ascontiguousarray(np.broadcast_to(
            nz[c * nw * P : (c + 1) * nw * P], (2, nw * P)))
        per_core.append({"xt": xt, "sfp8": sfp8, "rinv4": rinv4, "nz2": nz2})
    return per_core, nw, nb


def _host_consts(wts):
    b1 = _f16(wts["phi_b1"])
    # K=128 block-diagonal bias rhs: rows 0-3 carry b1 per chunk-block,
    # rows 4-127 are zero (so the lhsT rows 4-127 can hold garbage)
    b1diag = np.zeros((P, 512), dtype=np.float16)
    for k in range(4):
        b1diag[k, k * P : (k + 1) * P] = b1
    b2hi = _f16(wts["phi_b2"])
    b2lo = _f16(np.asarray(wts["phi_b2"], np.float32) - b2hi.astype(np.float32))
    cpack16 = np.concatenate(
        [_f16(wts["phi_W1"]), _f16(wts["phi_W2"]),
         _f16(wts["rho_W1"]), _f16(wts["rho_W2"]), b1diag], axis=1)
    cpack32 = np.concatenate(
        [np.asarray(wts["rho_b1"], np.float32).reshape(P, 1),
         np.asarray(wts["rho_b2"], np.float32).reshape(P, 1)], axis=1)
    return {
        "cpack16": cpack16,          # [128, 1024]: w1|w2|rw1|rw2|b1diag
        "cpack32": cpack32,          # [128, 2]:    rb1|rb2
        "b2c2": np.stack([b2hi, b2lo]),
    }


def _build(nw, nb, consts_np):
    """Emit the SPMD single-core program (same NEFF for all 8 cores)."""
    slots = nb * P
    nch = -(-nb // 4)
    nc = bacc.Bacc("TRN2", target_bir_lowering=False, debug=False,
                   num_devices=N_CORES)

    d_xt = nc.dram_tensor("xt", [P, nw * slots], F16, kind="ExternalInput").ap()
    d_s = nc.dram_tensor("sfp8", [P, nw * nb * P], F8,
                         kind="ExternalInput").ap()
    d_r4 = nc.dram_tensor("rinv4", [4, nw * nch * P], F16,
                          kind="ExternalInput").ap()
    d_nz2 = nc.dram_tensor("nz2", [2, nw * P], F16, kind="ExternalInput").ap()
    d_consts = {
        k: nc.dram_tensor(
            k, list(v.shape), mybir.dt.from_np(v.dtype), kind="ExternalInput"
        ).ap()
        for k, v in consts_np.items()
    }
    d_out = nc.dram_tensor("outT", [P, nw * P], F32, kind="ExternalOutput").ap()

    chunks = []
    off = 0
    while off < nb:
        cs = min(4, nb - off)
        chunks.append((off, cs))
        off += cs

    with tile.TileContext(nc) as tc:
        with (
            tc.tile_pool(name="const", bufs=1) as constp,
            tc.tile_pool(name="outsb", bufs=1) as outp,
            tc.tile_pool(name="xt", bufs=3) as xtp,
            tc.tile_pool(name="sfp", bufs=3) as sfpp,
            tc.tile_pool(name="h1r", bufs=2) as h1rp,
            tc.tile_pool(name="tail16", bufs=6) as tailp,
            tc.tile_pool(name="h1ps", bufs=3, space="PSUM") as h1psp,
            tc.tile_pool(name="tps", bufs=2, space="PSUM") as tpsp,
            tc.tile_pool(name="tailps", bufs=2, space="PSUM") as tailpsp,
        ):
            cs_ = {}
            for k, v in consts_np.items():
                cs_[k] = constp.tile(
                    list(v.shape), mybir.dt.from_np(v.dtype), name=f"c_{k}"
                )
                # scalar-engine HWDGE ring: parallel with the sync-ring
                # window DMAs so startup isn't serialized
                nc.scalar.dma_start(cs_[k], d_consts[k])
            w1_c = cs_["cpack16"][:, 0:128]
            w2_c = cs_["cpack16"][:, 128:256]
            rw1_c = cs_["cpack16"][:, 256:384]
            rw2_c = cs_["cpack16"][:, 384:512]
            b1diag_c = cs_["cpack16"][:, 512:1024]
            rb1_c = cs_["cpack32"][:, 0:1]
            rb2_c = cs_["cpack32"][:, 1:2]
            nz2sb = constp.tile([2, nw * P], F16)
            nc.scalar.dma_start(nz2sb, d_nz2)
            outsb = outp.tile([P, nw * P], F32)
            # ping-pong K=128 stationary for the bias rank-1s: rows 0-3 get
            # the per-window rinv values (tiny DMA, one window ahead),
            # rows 4-127 are zeroed once and multiply b1diag's zero rows
            padt = constp.tile([P, 2 * nch * P], F16, name="padt")
            nc.gpsimd.memset(padt, 0.0)

            def emit_rinv_copy(w):
                half = (w % 2) * nch * P
                nc.scalar.dma_start(
                    padt[0:4, half : half + nch * P],
                    d_r4[:, w * nch * P : (w + 1) * nch * P])

            # The K=4 bias rank-1 for each psum chunk is emitted one chunk
            # AHEAD of the block matmuls that accumulate onto it: a matmul
            # that accumulates onto the region a previous matmul just wrote
            # pays a full drain-before-fill bubble (~320ns); with >=4
            # unrelated matmuls in between the drain is hidden.
            def emit_bias(w, ci):
                coff, csz = chunks[ci]
                t = h1psp.tile([P, 512], F32, space="PSUM", tag="h1ps",
                               name=f"h1ps_{w}_{ci}")
                half = (w % 2) * nch * P
                nc.tensor.matmul(
                    t[:, : csz * P],
                    lhsT=padt[:, half + ci * P : half + (ci + 1) * P],
                    rhs=b1diag_c[:, : csz * P],
                    start=True, stop=False,
                )
                return t

            all_chunks = [(w, ci) for w in range(nw) for ci in range(nch)]
            gpos = 0
            emit_rinv_copy(0)
            nxt_tile = emit_bias(*all_chunks[0])

            # groups of up to 4 windows share one phi-L2 + rho tail (N=512
            # matmuls amortize the serial per-window tail latency)
            GW = 4
            for w0 in range(0, nw, GW):
                g = min(GW, nw - w0)
                t_sb = tailp.tile([P, g * P], F16, tag="t_sb",
                                  padded_shape=[P, GW * P])
                for w in range(w0, w0 + g):
                    if w + 1 < nw:
                        emit_rinv_copy(w + 1)
                    xt = xtp.tile([P, slots], F16)
                    nc.sync.dma_start(xt, d_xt[:, w * slots : (w + 1) * slots])
                    st = sfpp.tile([P, nb * P], F8)
                    nc.sync.dma_start(
                        st, d_s[:, w * nb * P : (w + 1) * nb * P])

                    # ---- phi layer 1: h1r = relu(Xs @ W1 + rinv*b1)
                    h1r = h1rp.tile([P, slots], F16)
                    for ci, (coff, csz) in enumerate(chunks):
                        h1ps = nxt_tile
                        if gpos + 1 < len(all_chunks):
                            nxt_tile = emit_bias(*all_chunks[gpos + 1])
                        gpos += 1
                        reg = h1ps[:, : csz * P]
                        for j in range(csz):
                            b = coff + j
                            nc.tensor.matmul(
                                h1ps[:, j * P : (j + 1) * P],
                                lhsT=xt[:, b * P : (b + 1) * P],
                                rhs=w1_c,
                                start=False, stop=(j == csz - 1),
                            )
                        dst = h1r[:, coff * P : (coff + csz) * P]
                        if ci in (0, 3):
                            nc.scalar.activation(
                                dst, reg, mybir.ActivationFunctionType.Relu)
                        else:
                            nc.vector.tensor_scalar(
                                dst, reg, 0.0, None, op0=mybir.AluOpType.max)

                    # ---- segment-mean reduce: T'[hid,seg] += h1r_b^T @ S_b
                    tps = tpsp.tile([P, P], F32, space="PSUM", tag="tps")
                    for b in range(nb):
                        nc.tensor.matmul(
                            tps, lhsT=h1r[:, b * P : (b + 1) * P],
                            rhs=st[:, b * P : (b + 1) * P],
                            start=(b == 0), stop=(b == nb - 1),
                        )
                    nc.scalar.copy(t_sb[:, (w - w0) * P : (w - w0 + 1) * P],
                                   tps)

                # ---- phi layer 2 on segment means (whole group):
                #      sm^T = W2^T @ T' + b2 x nz
                smps = tailpsp.tile([P, g * P], F32, space="PSUM",
                                    tag="tailps", padded_shape=[P, GW * P])
                nc.tensor.matmul(smps, lhsT=w2_c, rhs=t_sb,
                                 start=True, stop=False)
                nc.tensor.matmul(
                    smps, lhsT=cs_["b2c2"],
                    rhs=nz2sb[:, w0 * P : (w0 + g) * P],
                    start=False, stop=True,
                )
                sm_sb = tailp.tile([P, g * P], F16, tag="sm_sb",
                                   padded_shape=[P, GW * P])
                nc.vector.tensor_copy(sm_sb, smps)

                # ---- rho MLP (feature-major: per-partition ACT biases)
                r1ps = tailpsp.tile([P, g * P], F32, space="PSUM",
                                    tag="tailps", padded_shape=[P, GW * P])
                nc.tensor.matmul(r1ps, lhsT=rw1_c, rhs=sm_sb,
                                 start=True, stop=True)
                r1_sb = tailp.tile([P, g * P], F16, tag="r1_sb",
                                   padded_shape=[P, GW * P])
                nc.scalar.activation(
                    r1_sb, r1ps, mybir.ActivationFunctionType.Relu,
                    bias=rb1_c,
                )
                ops_ = tailpsp.tile([P, g * P], F32, space="PSUM",
                                    tag="tailps", padded_shape=[P, GW * P])
                nc.tensor.matmul(ops_, lhsT=rw2_c, rhs=r1_sb,
                                 start=True, stop=True)
                nc.scalar.activation(
                    outsb[:, w0 * P : (w0 + g) * P], ops_,
                    mybir.ActivationFunctionType.Identity,
                    bias=rb2_c,
                )
                nc.sync.dma_start(d_out[:, w0 * P : (w0 + g) * P],
                                  outsb[:, w0 * P : (w0 + g) * P])

    nc.compile()
    return nc


def _run(inputs, n_segs=50000, trace=False, **hw_kwargs):
    ins = np.asarray(inputs["ins"])
    batch = np.asarray(inputs["batch"])
    per_core, nw, nb = _host_prep(ins, batch, inputs, n_segs)
    consts_np = _host_consts(inputs)
    nc = _build(nw, nb, consts_np)

    in_maps = []
    for c in range(N_CORES):
        m = dict(consts_np)
        m.update(per_core[c])
        in_maps.append(m)
    res = run_bass_kernel_spmd(
        nc, in_maps, core_ids=list(range(N_CORES)), trace=trace, **hw_kwargs
    )
    outs = [r["outT"] for r in res.results]             # [128, nw*128] f32
    full = np.concatenate([o.T for o in outs], axis=0)  # [8*nw*128, 128]
    return np.ascontiguousarray(full[:n_segs]), res


def kernel(**inputs):
    out, _ = _run(inputs)
    return out


# revision 16
# speedup vs baseline: 2.0368x; 1.1017x over previous
"""DeepSets segment-reduce kernel for 8x TRN2 NeuronCores (Bass/Tile).

Computes: out = rho_mlp(segment_mean(phi_mlp(ins), batch))  for
sorted segment ids `batch` in [0, 50000), ins [1M, 128] f32.

Strategy:
  - Segments are grouped in windows of 128, windows assigned contiguously
    to the 8 cores (no segment straddles a core => zero cross-core
    collectives). One SPMD NEFF serves all cores; per-window row counts
    are padded to a uniform block count (pad rows contribute exactly 0).
  - Host preprocessing (numpy): transpose ins per window into fp16
    [128, slots] tiles with each row pre-scaled by 1/count(segment) (this
    folds the segment-mean into the phi layer-1 output, and zeroes pad
    rows), plus fp8 one-hot selection matrices S per 128-row block.
  - Device, per window:
      h1 = relu(Xs @ W1 + rinv*b1)   -- X^T block stationary, W1 moving;
                                        bias via K=4 block-diag rank-1
      T' = sum_b h1r_b^T-contracted  -- matmul lhsT=h1r_b, rhs=S_b (fp8)
           => T'[hid, seg] accumulated in PSUM = segment-MEANS of h1r
      seg_mean^T = W2^T @ T' + b2 x nz  (phi layer 2 reassociated onto
           50k segments instead of 1M rows, ~20x less work)
      rho MLP on seg_mean^T per window ([128,128] matmuls), biases via
      per-partition ACT bias in the transposed orientation.
  - All intermediates fp16 (full PE rate, values O(1)); end-to-end
    scale-relative absmax vs fp32 reference ~5e-4.

kernel(**inputs) takes the full unsharded inputs and returns the full
[50000, 128] fp32 output.
"""

import numpy as np
import ml_dtypes

import concourse.mybir as mybir
import concourse.tile as tile
from concourse import bacc
from concourse.bass_utils import run_bass_kernel_spmd

P = 128
N_CORES = 8
F16 = mybir.dt.float16
F32 = mybir.dt.float32
F8 = mybir.dt.float8e4
FP8NP = ml_dtypes.float8_e4m3


def _f16(a):
    return np.asarray(a, dtype=np.float32).astype(np.float16)


def _host_prep(ins, batch, wts, n_segs):
    """Shard rows by 128-segment windows; build per-core device arrays."""
    batch = np.asarray(batch).astype(np.int64)
    # Absorb the phi layer-1 bias into X: with u = W1^-T b1 (solved against
    # the fp16-rounded W1), (x + u) @ W1f == x @ W1f + b1 exactly, so no
    # per-chunk bias matmuls are needed on the PE. Falls back to rank-1
    # bias matmuls if W1 is ill-conditioned (|u| too large would amplify
    # the fp16 rounding of x + u).
    W1f = _f16(wts["phi_W1"]).astype(np.float64)
    b1d = np.asarray(wts["phi_b1"], np.float64)
    try:
        u = np.linalg.solve(W1f.T, b1d)
        use_u = bool(np.isfinite(u).all() and np.abs(u).max() < 64.0)
    except np.linalg.LinAlgError:
        u, use_u = np.zeros(P), False
    nwin_tot = -(-n_segs // P)                      # 391 for 50000
    nw = -(-nwin_tot // N_CORES)                    # windows per core (49)
    nwin_pad = nw * N_CORES                         # 392
    segs_pad = nwin_pad * P                         # 50176

    counts = np.bincount(batch, minlength=segs_pad).astype(np.float64)
    invc = np.where(counts > 0, 1.0 / np.maximum(counts, 1.0), 0.0)
    nz = (counts > 0).astype(np.float16)

    bounds = np.searchsorted(batch, np.arange(nwin_pad + 1) * P, side="left")
    win_cnt = np.diff(bounds)
    nb = max(1, int(-(-win_cnt.max() // P)))        # 128-row blocks per window
    slots = nb * P
    nch = -(-nb // 4)                               # psum chunks per window

    row_invc = invc[batch].astype(np.float32)
    ins = np.asarray(ins, dtype=np.float32)
    seg_ar = np.arange(P, dtype=np.int64)

    per_core = []
    for c in range(N_CORES):
        xt = np.zeros((P, nw * slots), dtype=np.float16)
        relpad = np.full((nw, slots), -1, dtype=np.int64)
        rinvpad = np.zeros((nw, slots), dtype=np.float32)
        for w in range(nw):
            g = c * nw + w
            s, e = bounds[g], bounds[g + 1]
            cnt = e - s
            if cnt == 0:
                continue
            # fp16 X^T, pre-scaled by 1/count so the segment reduction
            # directly produces means (pad rows scale to exactly 0)
            xrows = ins[s:e] + u if use_u else ins[s:e]
            xt[:, w * slots : w * slots + cnt] = \
                (xrows * row_invc[s:e, None]).T
            relpad[w, :cnt] = batch[s:e] - g * P
            rinvpad[w, :cnt] = row_invc[s:e]
        # fp8 one-hot S per block: S[r, s] = (rel[r] == s)
        oneh = (relpad.reshape(nw * nb, P)[:, :, None] == seg_ar)  # [blk,r,s]
        sfp8 = np.ascontiguousarray(
            oneh.transpose(1, 0, 2).reshape(P, nw * nb * P)
        ).astype(FP8NP)
        # K=4 rank-1 bias weights: rinv rows per chunk-block
        rinv4 = np.zeros((4, nw * nch * P), dtype=np.float16)
        r3 = rinvpad.reshape(nw, nb, P)
        for ci in range(nch):
            csz = min(4, nb - ci * 4)
            for k in range(csz):
                col = (np.arange(nw) * nch + ci)[:, None] * P + np.arange(P)
                rinv4[k, col.ravel()] = r3[:, ci * 4 + k, :].astype(
                    np.float16).ravel()
        nz2 = np.ascontiguousarray(np.broadcast_to(
            nz[c * nw * P : (c + 1) * nw * P], (2, nw * P)))
        m = {"xt": xt, "sfp8": sfp8, "nz2": nz2}
        if not use_u:
            m["rinv4"] = rinv4
        per_core.append(m)
    return per_core, nw, nb, use_u


def _host_consts(wts):
    b1 = _f16(wts["phi_b1"])
    # K=128 block-diagonal bias rhs: rows 0-3 carry b1 per chunk-block,
    # rows 4-127 are zero (so the lhsT rows 4-127 can hold garbage)
    b1diag = np.zeros((P, 512), dtype=np.float16)
    for k in range(4):
        b1diag[k, k * P : (k + 1) * P] = b1
    b2hi = _f16(wts["phi_b2"])
    b2lo = _f16(np.asarray(wts["phi_b2"], np.float32) - b2hi.astype(np.float32))
    cpack16 = np.concatenate(
        [_f16(wts["phi_W1"]), _f16(wts["phi_W2"]),
         _f16(wts["rho_W1"]), _f16(wts["rho_W2"]), b1diag], axis=1)
    cpack32 = np.concatenate(
        [np.asarray(wts["rho_b1"], np.float32).reshape(P, 1),
         np.asarray(wts["rho_b2"], np.float32).reshape(P, 1)], axis=1)
    return {
        "cpack16": cpack16,          # [128, 1024]: w1|w2|rw1|rw2|b1diag
        "cpack32": cpack32,          # [128, 2]:    rb1|rb2
        "b2c2": np.stack([b2hi, b2lo]),
    }


def _build(nw, nb, consts_np, use_u):
    """Emit the SPMD single-core program (same NEFF for all 8 cores)."""
    slots = nb * P
    nch = -(-nb // 4)
    nc = bacc.Bacc("TRN2", target_bir_lowering=False, debug=False,
                   num_devices=N_CORES)

    d_xt = nc.dram_tensor("xt", [P, nw * slots], F16, kind="ExternalInput").ap()
    d_s = nc.dram_tensor("sfp8", [P, nw * nb * P], F8,
                         kind="ExternalInput").ap()
    d_r4 = (None if use_u else
            nc.dram_tensor("rinv4", [4, nw * nch * P], F16,
                           kind="ExternalInput").ap())
    d_nz2 = nc.dram_tensor("nz2", [2, nw * P], F16, kind="ExternalInput").ap()
    d_consts = {
        k: nc.dram_tensor(
            k, list(v.shape), mybir.dt.from_np(v.dtype), kind="ExternalInput"
        ).ap()
        for k, v in consts_np.items()
    }
    d_out = nc.dram_tensor("outT", [P, nw * P], F32, kind="ExternalOutput").ap()

    chunks = []
    off = 0
    while off < nb:
        cs = min(4, nb - off)
        chunks.append((off, cs))
        off += cs

    with tile.TileContext(nc) as tc:
        with (
            tc.tile_pool(name="const", bufs=1) as constp,
            tc.tile_pool(name="outsb", bufs=1) as outp,
            tc.tile_pool(name="xt", bufs=3) as xtp,
            tc.tile_pool(name="sfp", bufs=3) as sfpp,
            tc.tile_pool(name="h1r", bufs=2) as h1rp,
            tc.tile_pool(name="tail16", bufs=6) as tailp,
            tc.tile_pool(name="h1ps", bufs=3, space="PSUM") as h1psp,
            tc.tile_pool(name="tps", bufs=2, space="PSUM") as tpsp,
            tc.tile_pool(name="tailps", bufs=2, space="PSUM") as tailpsp,
        ):
            cs_ = {}
            for k, v in consts_np.items():
                cs_[k] = constp.tile(
                    list(v.shape), mybir.dt.from_np(v.dtype), name=f"c_{k}"
                )
                # scalar-engine HWDGE ring: parallel with the sync-ring
                # window DMAs so startup isn't serialized
                nc.scalar.dma_start(cs_[k], d_consts[k])
            w1_c = cs_["cpack16"][:, 0:128]
            w2_c = cs_["cpack16"][:, 128:256]
            rw1_c = cs_["cpack16"][:, 256:384]
            rw2_c = cs_["cpack16"][:, 384:512]
            b1diag_c = cs_["cpack16"][:, 512:1024]
            rb1_c = cs_["cpack32"][:, 0:1]
            rb2_c = cs_["cpack32"][:, 1:2]
            nz2sb = constp.tile([2, nw * P], F16)
            nc.scalar.dma_start(nz2sb, d_nz2)
            outsb = outp.tile([P, nw * P], F32)
            # ping-pong K=128 stationary for the bias rank-1s: rows 0-3 get
            # the per-window rinv values (tiny DMA, one window ahead),
            # rows 4-127 are zeroed once and multiply b1diag's zero rows
            padt = None
            if not use_u:
                padt = constp.tile([P, 2 * nch * P], F16, name="padt")
                nc.gpsimd.memset(padt, 0.0)

            def emit_rinv_copy(w):
                if use_u:
                    return
                half = (w % 2) * nch * P
                nc.scalar.dma_start(
                    padt[0:4, half : half + nch * P],
                    d_r4[:, w * nch * P : (w + 1) * nch * P])

            # The K=4 bias rank-1 for each psum chunk is emitted one chunk
            # AHEAD of the block matmuls that accumulate onto it: a matmul
            # that accumulates onto the region a previous matmul just wrote
            # pays a full drain-before-fill bubble (~320ns); with >=4
            # unrelated matmuls in between the drain is hidden.
            def emit_bias(w, ci):
                coff, csz = chunks[ci]
                t = h1psp.tile([P, 512], F32, space="PSUM", tag="h1ps",
                               name=f"h1ps_{w}_{ci}")
                if not use_u:
                    half = (w % 2) * nch * P
                    nc.tensor.matmul(
                        t[:, : csz * P],
                        lhsT=padt[:, half + ci * P : half + (ci + 1) * P],
                        rhs=b1diag_c[:, : csz * P],
                        start=True, stop=False,
                    )
                return t

            all_chunks = [(w, ci) for w in range(nw) for ci in range(nch)]
            gpos = 0
            emit_rinv_copy(0)
            nxt_tile = emit_bias(*all_chunks[0])

            # groups of up to 4 windows share one phi-L2 + rho tail (N=512
            # matmuls amortize the serial per-window tail latency)
            GW = 4
            for w0 in range(0, nw, GW):
                g = min(GW, nw - w0)
                t_sb = tailp.tile([P, g * P], F16, tag="t_sb",
                                  padded_shape=[P, GW * P])
                for w in range(w0, w0 + g):
                    if w + 1 < nw:
                        emit_rinv_copy(w + 1)
                    xt = xtp.tile([P, slots], F16)
                    nc.sync.dma_start(xt, d_xt[:, w * slots : (w + 1) * slots])
                    st = sfpp.tile([P, nb * P], F8)
                    nc.sync.dma_start(
                        st, d_s[:, w * nb * P : (w + 1) * nb * P])

                    # ---- phi layer 1: h1r = relu(Xs @ W1 + rinv*b1)
                    h1r = h1rp.tile([P, slots], F16)
                    for ci, (coff, csz) in enumerate(chunks):
                        h1ps = nxt_tile
                        if gpos + 1 < len(all_chunks):
                            nxt_tile = emit_bias(*all_chunks[gpos + 1])
                        gpos += 1
                        reg = h1ps[:, : csz * P]
                        for j in range(csz):
                            b = coff + j
                            nc.tensor.matmul(
                                h1ps[:, j * P : (j + 1) * P],
                                lhsT=xt[:, b * P : (b + 1) * P],
                                rhs=w1_c,
                                start=use_u, stop=(j == csz - 1),
                            )
                        dst = h1r[:, coff * P : (coff + csz) * P]
                        if ci in (0, 3):
                            nc.scalar.activation(
                                dst, reg, mybir.ActivationFunctionType.Relu)
                        else:
                            nc.vector.tensor_scalar(
                                dst, reg, 0.0, None, op0=mybir.AluOpType.max)

                    # ---- segment-mean reduce: T'[hid,seg] += h1r_b^T @ S_b
                    tps = tpsp.tile([P, P], F32, space="PSUM", tag="tps")
                    for b in range(nb):
                        nc.tensor.matmul(
                            tps, lhsT=h1r[:, b * P : (b + 1) * P],
                            rhs=st[:, b * P : (b + 1) * P],
                            start=(b == 0), stop=(b == nb - 1),
                        )
                    nc.scalar.copy(t_sb[:, (w - w0) * P : (w - w0 + 1) * P],
                                   tps)

                # ---- phi layer 2 on segment means (whole group):
                #      sm^T = W2^T @ T' + b2 x nz
                smps = tailpsp.tile([P, g * P], F32, space="PSUM",
                                    tag="tailps", padded_shape=[P, GW * P])
                nc.tensor.matmul(smps, lhsT=w2_c, rhs=t_sb,
                                 start=True, stop=False)
                nc.tensor.matmul(
                    smps, lhsT=cs_["b2c2"],
                    rhs=nz2sb[:, w0 * P : (w0 + g) * P],
                    start=False, stop=True,
                )
                sm_sb = tailp.tile([P, g * P], F16, tag="sm_sb",
                                   padded_shape=[P, GW * P])
                nc.vector.tensor_copy(sm_sb, smps)

                # ---- rho MLP (feature-major: per-partition ACT biases)
                r1ps = tailpsp.tile([P, g * P], F32, space="PSUM",
                                    tag="tailps", padded_shape=[P, GW * P])
                nc.tensor.matmul(r1ps, lhsT=rw1_c, rhs=sm_sb,
                                 start=True, stop=True)
                r1_sb = tailp.tile([P, g * P], F16, tag="r1_sb",
                                   padded_shape=[P, GW * P])
                nc.scalar.activation(
                    r1_sb, r1ps, mybir.ActivationFunctionType.Relu,
                    bias=rb1_c,
                )
                ops_ = tailpsp.tile([P, g * P], F32, space="PSUM",
                                    tag="tailps", padded_shape=[P, GW * P])
                nc.tensor.matmul(ops_, lhsT=rw2_c, rhs=r1_sb,
                                 start=True, stop=True)
                nc.scalar.activation(
                    outsb[:, w0 * P : (w0 + g) * P], ops_,
                    mybir.ActivationFunctionType.Identity,
                    bias=rb2_c,
                )
                nc.sync.dma_start(d_out[:, w0 * P : (w0 + g) * P],
                                  outsb[:, w0 * P : (w0 + g) * P])

    nc.compile()
    return nc


def _run(inputs, n_segs=50000, trace=False, **hw_kwargs):
    ins = np.asarray(inputs["ins"])
    batch = np.asarray(inputs["batch"])
    per_core, nw, nb, use_u = _host_prep(ins, batch, inputs, n_segs)
    consts_np = _host_consts(inputs)
    nc = _build(nw, nb, consts_np, use_u)

    in_maps = []
    for c in range(N_CORES):
        m = dict(consts_np)
        m.update(per_core[c])
        in_maps.append(m)
    res = run_bass_kernel_spmd(
        nc, in_maps, core_ids=list(range(N_CORES)), trace=trace, **hw_kwargs
    )
    outs = [r["outT"] for r in res.results]             # [128, nw*128] f32
    full = np.concatenate([o.T for o in outs], axis=0)  # [8*nw*128, 128]
    return np.ascontiguousarray(full[:n_segs]), res


def kernel(**inputs):
    out, _ = _run(inputs)
    return out
